# revision 38
# baseline (speedup 1.0000x reference)
"""Trainium2 Bass kernel for Swin-style window attention (MegatronWindowAttention).

Per window w (49 tokens, dim 256, 8 heads x 32):
  qkv = x @ qkv_w.T + qkv_b ; attn = softmax(q*scale @ k.T + bias + mask[w%64])
  out = (attn @ v) @ proj_w.T + proj_b

Sharding: data-parallel over B_=4096 windows across 8 cores (512 windows each).

Device dataflow per core (4 blocks of 128 windows; window PAIRS of 98 tokens):
  - x.T bf16 arrives via DMA xbar transpose (host passes bf16 copy of x)
  - Q.T/K.T staged per 8-pair group: W-stationary matmuls (FWL 128-col weights)
  - V per pair in [token, channel] layout: x.T-chunk-stationary matmuls
    streaming Wv.T (slotted 33-wide per head, ones col for rowsum)
  - S.T per (head, pair): one matmul, lhsT = K.T 128-col span (both windows
    compact at partitions 0:98), rhs = Q.T 98 cols; exp on ACT; * exp(bias)
    table (zeros kill cross-window garbage quadrants) on DVE
  - AV per head: lhsT = pm 128-col slot (FWL), rhs = V_aug -> O + rowsum
  - normalize via DVE reciprocal+mul; 2 PE transposes -> O.T; proj 2 matmuls
"""

import os
import numpy as np
import ml_dtypes
from contextlib import ExitStack

KSTAGE = int(os.environ.get("KSTAGE", "5"))

import concourse.bass as bass
import concourse.tile as tile
import concourse.mybir as mybir
from concourse import bacc
from concourse import bass_utils

WH = WW = 7
NTOK = 49
DIM = 256
NH = 8
HD = 32
SCALE = HD ** -0.5
NCORES = 8
B_FULL = 4096
NW = 64
B_CORE = B_FULL // NCORES          # 512 windows per core
T_CORE = B_CORE * NTOK             # 25088 tokens
NBLOCK = 4
W_BLK = 128                        # windows per block
T_BLK = W_BLK * NTOK               # 6272 tokens per block
T_PAD = T_BLK                      # no halo needed (98-col stationary reads)
NPAIR = W_BLK // 2                 # 64 pairs per block
GPAIR = 8                          # pairs per QK staging group
NGRP = NPAIR // GPAIR              # 8 groups per block
GW = GPAIR * 98                    # 784 tokens per group
GWH = GW                           # no halo needed (98-col stationary reads)
GCH = GWH // 2                     # 392: QK psum chunk width

F32 = mybir.dt.float32
BF16 = mybir.dt.bfloat16
AF = mybir.ActivationFunctionType
ALU = mybir.AluOpType


def _rel_pos_index():
    coords = np.stack(np.meshgrid(np.arange(WH), np.arange(WW), indexing='ij'))
    flat = coords.reshape(2, -1)
    rel = flat[:, :, None] - flat[:, None, :]
    rel = rel.transpose(1, 2, 0).copy()
    rel[:, :, 0] += WH - 1
    rel[:, :, 1] += WW - 1
    rel[:, :, 0] *= 2 * WW - 1
    return rel.sum(-1).reshape(-1)


def build_kernel(ctx: ExitStack, tc: tile.TileContext, ins: dict, out_ap: bass.AP,
                 mask_zero: bool, pb_zero: bool):
    nc = tc.nc
    xbf = ins["xbf"]        # [T_CORE, 256] bf16
    qkw = ins["qkw"]        # [128, 2, 4, 128] bf16 (ic, kb, ob(q0 q1 k0 k1), oc)
    wv = ins["wv"]          # [128, 2, 264] bf16 (ic, kb, slotted oc)
    pwt = ins["pwt"]        # [128, 2, 256] bf16
    ebias = ins["ebias"]    # [128, 8, 98] bf16 (fast) or placeholder
    idb = ins["identb"]     # [128, 128] bf16
    qkvb = ins["qkvb"]      # [128, 4] f32 per-partition bias for q0 q1 k0 k1
    vbb = ins["vbb"]        # [128, 264] f32 broadcast v-bias (slotted)
    expbm = ins.get("expbm")  # [128, 32*784] bf16 (general path only)
    pbb = ins.get("pbb")    # [128, 256] f32 (general path only)

    const = ctx.enter_context(tc.tile_pool(name="const", bufs=1))
    qkw_sb = const.tile([128, 2 * 4 * 128], BF16, tag="qkw")
    nc.sync.dma_start(qkw_sb[:], qkw.rearrange("p a b c -> p (a b c)"))
    qkw_v = qkw_sb[:].rearrange("p (a b c) -> p a b c", a=2, b=4)
    wv_sb = const.tile([128, 2 * 264], BF16, tag="wv")
    nc.sync.dma_start(wv_sb[:], wv.rearrange("p a c -> p (a c)"))
    wv_v = wv_sb[:].rearrange("p (a c) -> p a c", a=2)
    pwt_sb = const.tile([128, 2 * 256], BF16, tag="pwt")
    nc.sync.dma_start(pwt_sb[:], pwt.rearrange("p a c -> p (a c)"))
    idb_sb = const.tile([128, 128], BF16, tag="identb")
    nc.sync.dma_start(idb_sb[:], idb[:])
    qkvb_sb = const.tile([128, 4], F32, tag="qkvb")
    nc.sync.dma_start(qkvb_sb[:], qkvb[:])
    vbb_sb = const.tile([128, 264], F32, tag="vbb")
    nc.sync.dma_start(vbb_sb[:], vbb[:])
    if mask_zero:
        eb_sb = const.tile([128, 8 * 98], BF16, tag="ebias")
        nc.sync.dma_start(eb_sb[:], ebias.rearrange("p a c -> p (a c)"))
    else:
        eb_sb = const.tile([128, 32 * 784], BF16, tag="expbm")
        nc.sync.dma_start(eb_sb[:], expbm[:])
    if not pb_zero:
        pbb_sb = const.tile([128, 256], F32, tag="pbb")
        nc.sync.dma_start(pbb_sb[:], pbb[:])

    xt_pool = ctx.enter_context(tc.tile_pool(name="xt", bufs=2))
    qk_pool = ctx.enter_context(tc.tile_pool(name="qk", bufs=2))
    pm_pool = ctx.enter_context(tc.tile_pool(name="pm", bufs=2))
    v_pool = ctx.enter_context(tc.tile_pool(name="vsb", bufs=2))
    r_pool = ctx.enter_context(tc.tile_pool(name="rsb", bufs=2))
    on_pool = ctx.enter_context(tc.tile_pool(name="onorm", bufs=2))
    ot_pool = ctx.enter_context(tc.tile_pool(name="otsb", bufs=2))
    out_pool = ctx.enter_context(tc.tile_pool(name="outsb", bufs=3))

    # 8 PSUM banks: qkp 2 (also proj), sps 4 (2 banks x 2 bufs, also O.T), work 2
    ps_qk = ctx.enter_context(tc.tile_pool(name="psqk", bufs=2, space="PSUM"))
    ps_s = ctx.enter_context(tc.tile_pool(name="pss", bufs=1, space="PSUM"))
    ps_w = ctx.enter_context(tc.tile_pool(name="psw", bufs=2, space="PSUM"))


    # back half of a pair: AV, normalize, O.T, proj, output. Emitted one pair
    # late so its PE work overlaps the next pair's exp on ACT.
    prev = None

    def emit_back(st):
        pm, vsb, dst = st["pm"], st["vsb"], st["dst"]
        avw = ps_w.tile([128, 512], F32, tag="work", name="av_" + st["tag"])
        for h in range(NH):
            s_h = 2 * (h % 4) + h // 4
            nc.tensor.matmul(
                avw[0:98, 33 * h:33 * h + 33],
                pm[0:98, 98 * s_h:98 * s_h + 98],
                vsb[0:98, 33 * h:33 * h + 33], start=True, stop=True)
        av_v = avw[:, 0:264].rearrange("p (h c) -> p h c", h=8)
        recip = r_pool.tile([128, 8], F32, tag="recip")
        nc.vector.reciprocal(recip[0:98, :], av_v[0:98, :, 32])
        onorm = on_pool.tile([128, 256], BF16, tag="onorm")
        onorm_v = onorm[:].rearrange("p (h c) -> p h c", h=8)
        recip_b = recip[0:98, :].unsqueeze(2).broadcast_to([98, 8, 32])
        nc.vector.tensor_mul(onorm_v[0:98, :, :], av_v[0:98, :, 0:32], recip_b)
        # O.T via 2 PE transposes -> work psum -> sbuf (98-packed slots)
        otw = ps_w.tile([128, 512], F32, tag="work", name="ot_" + st["tag"])
        otp = otw[:].bitcast(BF16)
        for hb in range(2):
            nc.tensor.transpose(
                otp[:, 98 * hb:98 * hb + 98],
                onorm[0:98, 128 * hb:128 * (hb + 1)],
                idb_sb[0:98, 0:98])
        otsb = ot_pool.tile([128, 196], BF16, tag="otsb")
        otsb_v = otsb[:].rearrange("p (b c) -> p b c", b=2)
        otp_v = otp[:, 0:196].rearrange("p (b c) -> p b c", b=2)
        nc.scalar.copy(otsb_v, otp_v)
        # proj (lhsT slots packed 98 apart, 128-wide reads); psum from the
        # qkp pool, which is idle during the pair stream
        pw4 = ps_qk.tile([128, 512], F32, tag="qkp", name="pj_" + st["tag"])
        for hb in range(2):
            nc.tensor.matmul(pw4[0:98, 0:256], otsb[:, 98 * hb:98 * hb + 98],
                             pwt_sb[:, 256 * hb:256 * (hb + 1)],
                             start=(hb == 0), stop=(hb == 1))
        osb = out_pool.tile([128, 256], F32, tag="outsb")
        if pb_zero:
            nc.scalar.copy(osb[0:98, :], pw4[0:98, 0:256])
        else:
            nc.vector.tensor_add(osb[0:98, :], pw4[0:98, 0:256], pbb_sb[0:98, :])
        nc.sync.dma_start(dst, osb[0:98, :])

    for blk in range(NBLOCK):
        t0 = blk * T_BLK
        # ---- x.T loaded directly (host pre-transposed); zero the tail pad ----
        Xt = [xt_pool.tile([128, T_PAD], BF16, tag=f"xt{kb}", name=f"xt{kb}_{blk}")
              for kb in range(2)]
        for kb in range(2):
            nc.sync.dma_start(Xt[kb][:, 0:T_BLK], xbf[kb, :, t0:t0 + T_BLK])
        for grp in range(NGRP):
            g0 = GW * grp
            # ---- Q.T/K.T staging for this group (halo of 32 for K reads) ----
            # last group's halo reads the zeroed pad region
            qks = [qk_pool.tile([128, GWH], BF16, tag=f"qk{ob}", name=f"qk{ob}_{blk}_{grp}")
                   for ob in range(4)]
            for c2 in range(2):
                cs = slice(g0 + GCH * c2, g0 + GCH * (c2 + 1))
                for ob in range(4):
                    qkp = ps_qk.tile([128, 512], F32, tag="qkp")
                    for kb in range(2):
                        nc.tensor.matmul(qkp[:, 0:GCH], qkw_v[:, kb, ob, :],
                                         Xt[kb][:, cs],
                                         start=(kb == 0), stop=(kb == 1))
                    nc.vector.tensor_scalar_add(
                        qks[ob][:, GCH * c2:GCH * (c2 + 1)], qkp[:, 0:GCH],
                        qkvb_sb[:, ob:ob + 1])
            # ---- attention pairs (software-skewed pipeline) ----
            for p8 in range(GPAIR):
                u = GPAIR * grp + p8
                c0 = 98 * u            # block-local token offset
                cg = 98 * p8           # group-local token offset
                # ---- front half of pair u: S, V, exp, pm ----
                # S.T: one matmul per head, both windows (keys compact 0:98).
                # Concurrent row-bands must use distinct PSUM banks: band
                # h%4 -> bank h%4; heads h, h+4 share a band (serialized by
                # the PE) and pack side by side within the bank.
                sps = ps_s.tile([128, 2048], F32, tag="sps")
                for h in range(NH):
                    kt = qks[2 + h // 4]
                    qt = qks[0 + h // 4]
                    rs = slice(32 * (h % 4), 32 * (h % 4) + 32)
                    sc = 512 * (h % 4) + 98 * (h // 4)
                    nc.tensor.matmul(
                        sps[0:98, sc:sc + 98],
                        kt[rs, cg:cg + 98], qt[rs, cg:cg + 98],
                        start=True, stop=True, tile_position=(32 * (h % 4), 0))
                # V in [token, channel-slot] layout: x.T chunk stationary
                vw = ps_w.tile([128, 512], F32, tag="work", name=f"v_{blk}_{u}")
                for kb in range(2):
                    nc.tensor.matmul(vw[0:98, 0:264], Xt[kb][:, c0:c0 + 98],
                                     wv_v[:, kb, :], start=(kb == 0), stop=(kb == 1))
                vsb = v_pool.tile([128, 264], BF16, tag="vsb")
                nc.vector.tensor_add(vsb[0:98, :], vw[0:98, 0:264], vbb_sb[0:98, :])
                vsb_v = vsb[:].rearrange("p (h c) -> p h c", h=8)
                nc.gpsimd.memset(vsb_v[0:98, :, 32:33], 1.0)
                # exp (ACT) then * exp(bias[+mask]) (DVE, zeros kill garbage).
                # pm slot s = 2*(h%4) + h//4, packed 98 apart: AV's 128-wide
                # lhsT reads spill into the next slot's valid data (garbage ->
                # unused out partitions 98:128); last slot gets a tail memset.
                pex = pm_pool.tile([128, 784], BF16, tag="pex")
                pex_v = pex[:].rearrange("p (r c) -> p r c", r=4)[0:98]
                sps_v = sps[:].rearrange("p (r c) -> p r c", r=4)[0:98, :, 0:196]
                nc.scalar.activation(pex_v, sps_v, AF.Exp)
                pm = pm_pool.tile([128, 784], BF16, tag="pm")
                pm_v = pm[:].rearrange("p (r c) -> p r c", r=4)[0:98]
                if mask_zero:
                    ebv = eb_sb[:].rearrange("p (r c) -> p r c", r=4)[0:98]
                else:
                    ebv = eb_sb[:, 784 * (u % 32):784 * (u % 32 + 1)].rearrange(
                        "p (r c) -> p r c", r=4)[0:98]
                nc.gpsimd.tensor_mul(pm_v, pex_v, ebv)
                # ---- back half of the PREVIOUS pair ----
                if prev is not None:
                    emit_back(prev)
                prev = dict(pm=pm, vsb=vsb, dst=out_ap[t0 + c0:t0 + c0 + 98, :],
                            tag=f"{blk}_{u}")
    if prev is not None:
        emit_back(prev)


def _noop():
    pass


_CACHED = {}


def _get_program(mask_zero: bool, pb_zero: bool):
    key = (mask_zero, pb_zero)
    if key in _CACHED:
        return _CACHED[key]
    nc = bacc.Bacc("TRN2", target_bir_lowering=False, debug=False)
    ins = {
        "xbf": nc.dram_tensor("xbf", [2, 128, T_CORE], BF16, kind="ExternalInput").ap(),
        "qkw": nc.dram_tensor("qkw", [128, 2, 4, 128], BF16, kind="ExternalInput").ap(),
        "wv": nc.dram_tensor("wv", [128, 2, 264], BF16, kind="ExternalInput").ap(),
        "pwt": nc.dram_tensor("pwt", [128, 2, 256], BF16, kind="ExternalInput").ap(),
        "ebias": nc.dram_tensor("ebias", [128, 8, 98], BF16, kind="ExternalInput").ap(),
        "identb": nc.dram_tensor("identb", [128, 128], BF16, kind="ExternalInput").ap(),
        "qkvb": nc.dram_tensor("qkvb", [128, 4], F32, kind="ExternalInput").ap(),
        "vbb": nc.dram_tensor("vbb", [128, 264], F32, kind="ExternalInput").ap(),
    }
    if not mask_zero:
        ins["expbm"] = nc.dram_tensor("expbm", [128, 32 * 784], BF16,
                                      kind="ExternalInput").ap()
    if not pb_zero:
        ins["pbb"] = nc.dram_tensor("pbb", [128, 256], F32, kind="ExternalInput").ap()
    out_ap = nc.dram_tensor("out", [T_CORE, DIM], F32, kind="ExternalOutput").ap()
    with tile.TileContext(nc) as tc:
        with ExitStack() as ctx:
            build_kernel(ctx, tc, ins, out_ap, mask_zero, pb_zero)
    nc.compile()
    _CACHED[key] = nc
    return nc


def _host_prep(mask, qkv_w, qkv_b, proj_w, proj_b, bias_table):
    bf = ml_dtypes.bfloat16
    qkv_w = np.asarray(qkv_w, np.float32)
    qkv_b = np.asarray(qkv_b, np.float32)
    mask = np.asarray(mask, np.float32)
    mask_zero = not np.any(mask)
    pb = np.asarray(proj_b, np.float32)
    pb_zero = not np.any(pb)

    wqk = qkv_w[0:512].copy()          # [512 oc, 256 ic]
    wqk[0:256] *= SCALE                # fold softmax scale into q
    # [ic, oc] -> [kb, 128ic, ob, 128oc] -> [128ic, kb, ob, 128oc]
    qkw = np.ascontiguousarray(
        wqk.T.reshape(2, 128, 4, 128).transpose(1, 0, 2, 3)).astype(bf)
    qb = qkv_b.copy()
    qb[0:256] *= SCALE
    qkvb = np.ascontiguousarray(qb[0:512].reshape(4, 128).T)   # [128, 4]

    wvT = qkv_w[512:768].T             # [256 ic, 256 oc]
    wv = np.zeros((2, 128, 264), np.float32)
    for h in range(NH):
        wv[:, :, 33 * h:33 * h + 32] = wvT.reshape(2, 128, 8, 32)[:, :, h]
    wv = np.ascontiguousarray(wv.transpose(1, 0, 2)).astype(bf)
    vb = np.zeros((264,), np.float32)
    for h in range(NH):
        vb[33 * h:33 * h + 32] = qkv_b[512 + 32 * h:512 + 32 * h + 32]
    vbb = np.ascontiguousarray(np.broadcast_to(vb, (128, 264)))

    pwt = np.ascontiguousarray(
        np.asarray(proj_w, np.float32).T.reshape(2, 128, 256)
        .transpose(1, 0, 2)).astype(bf)
    pbb = np.ascontiguousarray(np.broadcast_to(pb, (128, 256)))

    rel = _rel_pos_index()
    bias_g = np.asarray(bias_table, np.float32)[rel].reshape(NTOK, NTOK, NH)  # [i,j,h]
    ebT = np.exp(bias_g).transpose(1, 2, 0)          # [j, h, i]
    # device pm slot s = 2*(h%4) + h//4 -> head order [0,4,1,5,2,6,3,7]
    SLOT_ORDER = [0, 4, 1, 5, 2, 6, 3, 7]
    ebias = np.zeros((128, 8, 98), np.float32)
    ebias[0:49, :, 0:49] = ebT
    ebias[49:98, :, 49:98] = ebT
    ebias = np.ascontiguousarray(ebias[:, SLOT_ORDER, :]).astype(bf)

    expbm = None
    if not mask_zero:
        expbm = np.zeros((128, 32, 8, 98), np.float32)
        for p in range(32):
            for w in range(2):
                cb = np.exp(bias_g + mask[2 * p + w][:, :, None]).transpose(1, 2, 0)
                expbm[49 * w:49 * w + 49, p, :, 49 * w:49 * w + 49] = cb
        expbm = np.ascontiguousarray(
            expbm[:, :, SLOT_ORDER, :].reshape(128, 32 * 784)).astype(bf)
    identb = np.eye(128).astype(bf)
    return (qkw, qkvb, wv, vbb, pwt, pbb, ebias, expbm, identb,
            mask_zero, pb_zero)


def kernel(x, mask, qkv_w, qkv_b, proj_w, proj_b, bias_table, _trace=False):
    bf = ml_dtypes.bfloat16
    (qkw, qkvb, wv, vbb, pwt, pbb, ebias, expbm, identb,
     mask_zero, pb_zero) = _host_prep(mask, qkv_w, qkv_b, proj_w, proj_b, bias_table)
    # [T_full, 256] -> per-core [2, 128, T_CORE] bf16 (x.T, kb-major)
    xT = np.asarray(x, np.float32).reshape(B_FULL * NTOK, DIM).T.astype(bf)
    xTs = xT.reshape(2, 128, NCORES, T_CORE)
    in_maps = []
    for c in range(NCORES):
        shard = np.ascontiguousarray(xTs[:, :, c, :])
        m = {"xbf": shard, "qkw": qkw, "wv": wv, "pwt": pwt, "ebias": ebias,
             "identb": identb, "qkvb": qkvb, "vbb": vbb}
        if not mask_zero:
            m["expbm"] = expbm
        if not pb_zero:
            m["pbb"] = pbb
        in_maps.append(m)
    nc = _get_program(mask_zero, pb_zero)
    res = bass_utils.run_bass_kernel_spmd(nc, in_maps, core_ids=list(range(NCORES)),
                                          trace=_trace)
    out = np.stack([r["out"] for r in res.results])  # [8, T_CORE, 256]
    out = out.reshape(B_FULL, NTOK, DIM)
    if _trace:
        kernel.last_results = res
    return out
